# revision 4
# baseline (speedup 1.0000x reference)
"""Windowed BCJR detector kernel for Trainium2, 8-core batch-parallel.

Layout per core: 128 words on SBUF partitions.  Time axis split into
C = T/L chunks of L=32 steps, each warmed up with W=16 extra steps from
a uniform state (chunk 0 / last chunk get the exact delta init); all
chunks advance in lockstep so each recursion step is one wide vector op
pair instead of 2*T tiny ones.  Chunks are split DVE/Pool by a tunable
boundary so both engines run chains concurrently.

g is built as exp(scale*(y-sp)^2): squares via Act Square (per-state
bias) plus DVE/Pool tensor ops for the early tau range, then exp in
tau-quarter blocks ordered so the alpha warmup unblocks first.
g layout is tau-major ((tau*C + c)*16 + s).

Combine (u = x*beta, even-odd diff, tree-sum, sign) trails the beta
chain in 8-tau slices against a 16-slot beta ring.

States are bf16 without intra-window rescaling: decisions are invariant
to per-(word,t) positive scaling, and a window whose likelihood mass
underflows decodes 0 exactly like the reference's NaN cascade.
"""

import math
import sys

import numpy as np

sys.path.insert(0, "/opt/trn_rl_repo")

B, T, S, MEM, V = 1024, 2048, 16, 4, 4
NCORES = 8
BPC = B // NCORES   # 128 words per core
L = 32              # chunk length (t-steps)
W = 16              # warmup steps
SLC = 8             # combine slice (tau steps)
RING = 16           # beta ring slots
HS = S // 2
CDF = 30 / 64       # fraction of chunks on DVE (alpha-heavy on DVE)


def _build(nc, Tn, g_scale):
    import concourse.bass as bass  # noqa: F401
    from concourse import mybir, tile
    from concourse.alu_op_type import AluOpType as OP
    from concourse.mybir import ActivationFunctionType as AF

    dt = mybir.dt
    C = Tn // L
    CD = max(1, int(round(C * CDF)))
    CP = C - CD
    assert CP >= 1

    yin_d = nc.dram_tensor("yin", [BPC, Tn + S], dt.float32, kind="ExternalInput")
    out_d = nc.dram_tensor("dec", [BPC, Tn], dt.float32, kind="ExternalOutput")

    with tile.TileContext(nc) as tc:
        with tc.tile_pool(name="per", bufs=1) as per:
            spneg = per.tile([BPC, S], dt.float32, tag="spneg")
            zero1 = per.tile([BPC, 1], dt.float32, tag="zero1")
            gbuf = per.tile([BPC, Tn * S], dt.bfloat16, tag="g")
            dec = per.tile([BPC, Tn], dt.float32, tag="dec")

            # ---------------- g generation ----------------
            with tc.tile_pool(name="gg", bufs=1) as gg:
                ybuf = gg.tile([BPC, Tn + S], dt.float32, tag="y")
                sq = gg.tile([BPC, Tn * S], dt.float16, tag="sq")
                dtmpD = gg.tile([BPC, SLC * C * S], dt.float16, tag="dtD")
                dtmpP = gg.tile([BPC, (W - SLC) * C * S], dt.float16, tag="dtP")
                nc.sync.dma_start(ybuf[:, :], yin_d[:, :])
                nc.vector.tensor_copy(spneg[:, :], ybuf[:, Tn : Tn + S])
                nc.vector.memset(zero1[:, :], 0.0)

                def y_wc(w0, n):
                    """y view [p, w, c]: element (w, c) = y[c*L + w0 + w]."""
                    return ybuf[:, 0 : C * L].rearrange(
                        "p (c w) -> p w c", c=C, w=L
                    )[:, w0 : w0 + n, :]

                def sq_dp(eng, dtmp, w0, w1):
                    """sq rows tau [w0,w1) via d = y - sp; sq = d*d."""
                    n = w1 - w0
                    dv = dtmp[:, : n * C * S].rearrange(
                        "p (w c s) -> p w c s", w=n, c=C, s=S
                    )
                    yv = y_wc(w0, n).unsqueeze(3).broadcast_to((BPC, n, C, S))
                    sv = (
                        spneg[:, :]
                        .unsqueeze(1)
                        .unsqueeze(1)
                        .broadcast_to((BPC, n, C, S))
                    )
                    eng.tensor_tensor(dv, yv, sv, OP.add)
                    flat = dtmp[:, : n * C * S]
                    eng.tensor_tensor(
                        sq[:, w0 * C * S : w1 * C * S], flat, flat, OP.mult
                    )

                def sq_act(w0, w1):
                    """sq rows tau [w0,w1) via Act Square, one call per s."""
                    n = w1 - w0
                    sq4 = sq[:, :].rearrange(
                        "p (t c s) -> p t c s", c=C, s=S
                    )
                    for s in range(S):
                        nc.scalar.activation(
                            sq4[:, w0:w1, :, s],
                            y_wc(w0, n),
                            AF.Square,
                            bias=spneg[:, s : s + 1],
                            scale=1.0,
                        )

                sq_dp(nc.vector, dtmpD, 0, SLC)
                sq_dp(nc.gpsimd, dtmpP, SLC, W)
                sq_act(W, L)

                # exp quarters ordered q3 q4 q1 q2 (alpha warmup reads
                # tau [L-W, L) first, then the main sweep from tau 0).
                QT = L // 4
                for q in (2, 3, 0, 1):
                    o0 = q * QT * C * S
                    o1 = (q + 1) * QT * C * S
                    nc.scalar.activation(
                        gbuf[:, o0:o1], sq[:, o0:o1], AF.Exp,
                        bias=zero1[:, :], scale=float(g_scale),
                    )

            # ---------------- chains + combine ----------------
            with tc.tile_pool(name="ch", bufs=1) as ch:
                xD = ch.tile([BPC, L * CD * S], dt.bfloat16, tag="xD")
                xP = ch.tile([BPC, L * CP * S], dt.bfloat16, tag="xP")
                rgD = ch.tile([BPC, RING * CD * S], dt.bfloat16, tag="rD")
                rgP = ch.tile([BPC, RING * CP * S], dt.bfloat16, tag="rP")
                stDA = ch.tile([BPC, CD * S], dt.bfloat16, tag="stDA")
                stDB = ch.tile([BPC, CD * S], dt.bfloat16, tag="stDB")
                stPA = ch.tile([BPC, CP * S], dt.bfloat16, tag="stPA")
                stPB = ch.tile([BPC, CP * S], dt.bfloat16, tag="stPB")
                pD = ch.tile([BPC, CD * HS], dt.bfloat16, tag="pD")
                pP = ch.tile([BPC, CP * HS], dt.bfloat16, tag="pP")
                uD = ch.tile([BPC, SLC * CD * S], dt.bfloat16, tag="uD")
                uP = ch.tile([BPC, SLC * CP * S], dt.bfloat16, tag="uP")
                dcD = ch.tile([BPC, SLC * CD * HS], dt.bfloat16, tag="dcD")
                dcP = ch.tile([BPC, SLC * CP * HS], dt.bfloat16, tag="dcP")
                t4D = ch.tile([BPC, SLC * CD * 4], dt.bfloat16, tag="t4D")
                t4P = ch.tile([BPC, SLC * CP * 4], dt.bfloat16, tag="t4P")
                t2D = ch.tile([BPC, SLC * CD * 2], dt.bfloat16, tag="t2D")
                t2P = ch.tile([BPC, SLC * CP * 2], dt.bfloat16, tag="t2P")
                eD = ch.tile([BPC, SLC * CD], dt.bfloat16, tag="eD")
                eP = ch.tile([BPC, SLC * CP], dt.bfloat16, tag="eP")

                def v3(flat, n):
                    return flat[:, 0 : n * S].rearrange("p (c s) -> p c s", s=S)

                def g_view(tau, c0, c1):
                    o = (tau * C + c0) * S
                    return v3(gbuf[:, o : o + (c1 - c0) * S], c1 - c0)

                def alpha_add(eng, src3, ptile, n):
                    """p[c,j] = a[c,j] + a[c,j+8] (packed halves)."""
                    pv = ptile[:, : n * HS].rearrange("p (c j) -> p c j", j=HS)
                    eng.tensor_tensor(
                        pv, src3[:, :, 0:HS], src3[:, :, HS:S], OP.add
                    )

                def alpha_mult(eng, ptile, dst3, gv, n):
                    """x'[c, 2j+d] = p[c,j] * g[c, 2j+d] (p broadcast)."""
                    pv = (
                        ptile[:, : n * HS]
                        .rearrange("p (c j) -> p c j", j=HS)
                        .unsqueeze(3)
                        .broadcast_to((BPC, n, HS, 2))
                    )
                    eng.tensor_tensor(
                        dst3.rearrange("p c (j d) -> p c j d", d=2),
                        pv,
                        gv.rearrange("p c (j d) -> p c j d", d=2),
                        OP.mult,
                    )

                def beta_add(eng, src3, ptile, n):
                    """c[c,j] = b[c,2j] + b[c,2j+1] (stride-2)."""
                    s4 = src3.rearrange("p c (j d) -> p c j d", d=2)
                    cv = ptile[:, : n * HS].rearrange("p (c j) -> p c j", j=HS)
                    eng.tensor_tensor(cv, s4[:, :, :, 0], s4[:, :, :, 1], OP.add)

                def beta_mult(eng, ptile, dst3, gv, n):
                    """b'[c, d*8+j] = c[c,j] * g[c, d*8+j] (dup-block c)."""
                    cv = (
                        ptile[:, : n * HS]
                        .rearrange("p (c j) -> p c j", j=HS)
                        .unsqueeze(2)
                        .broadcast_to((BPC, n, 2, HS))
                    )
                    eng.tensor_tensor(
                        dst3.rearrange("p c (d j) -> p c d j", d=2),
                        cv,
                        gv.rearrange("p c (d j) -> p c d j", d=2),
                        OP.mult,
                    )

                def x_col(tau, gi):
                    xs, n = (xD, CD) if gi == 0 else (xP, CP)
                    return v3(xs[:, tau * n * S : (tau + 1) * n * S], n)

                def ring_col(tau, gi):
                    r = tau % RING
                    rs, n = (rgD, CD) if gi == 0 else (rgP, CP)
                    return v3(rs[:, r * n * S : (r + 1) * n * S], n)

                GRP = [
                    (nc.vector, 0, CD, stDA, stDB, pD),
                    (nc.gpsimd, CD, C, stPA, stPB, pP),
                ]

                # ---- alpha ----
                nc.vector.memset(stDA[:, :], 1.0)
                nc.gpsimd.memset(stPA[:, :], 1.0)
                flip = False
                for w in range(W):
                    for eng, c0, c1, sa, sb, pt in GRP:
                        cur, nxt = (sb, sa) if flip else (sa, sb)
                        cc0 = max(c0, 1)  # chunk 0 overridden below
                        lo = cc0 - c0
                        src = v3(cur[:, lo * S :], c1 - cc0)
                        alpha_add(eng, src, pt, c1 - cc0)
                        gv = g_view(L - W + w, cc0 - 1, c1 - 1)
                        alpha_mult(
                            eng, pt, v3(nxt[:, lo * S :], c1 - cc0), gv,
                            c1 - cc0,
                        )
                    flip = not flip
                curD = stDB if flip else stDA
                curP = stPB if flip else stPA
                # chunk 0 exact init: delta(state 0)
                nc.vector.memset(curD[:, 0:S], 0.0)
                nc.vector.memset(curD[:, 0:1], 1.0)
                for tau in range(L):
                    for gi, (eng, c0, c1, sa, sb, pt) in enumerate(GRP):
                        if tau == 0:
                            src = v3((curD if gi == 0 else curP)[:, :], c1 - c0)
                        else:
                            src = x_col(tau - 1, gi)
                        alpha_add(eng, src, pt, c1 - c0)
                        alpha_mult(
                            eng, pt, x_col(tau, gi), g_view(tau, c0, c1),
                            c1 - c0,
                        )

                # ---- beta (+ trailing combine) ----
                nc.vector.memset(stDA[:, :], 1.0)
                nc.gpsimd.memset(stPA[:, :], 1.0)
                flip = False
                for w in range(W):
                    for eng, c0, c1, sa, sb, pt in GRP:
                        cur, nxt = (sb, sa) if flip else (sa, sb)
                        cc1 = min(c1, C - 1)  # last chunk overridden below
                        src = v3(cur[:, :], cc1 - c0)
                        beta_add(eng, src, pt, cc1 - c0)
                        gv = g_view(W - 1 - w, c0 + 1, cc1 + 1)
                        beta_mult(
                            eng, pt, v3(nxt[:, :], cc1 - c0), gv, cc1 - c0
                        )
                    flip = not flip
                curD = stDB if flip else stDA
                curP = stPB if flip else stPA
                nc.gpsimd.memset(curP[:, (CP - 1) * S :], 0.0)
                nc.gpsimd.memset(curP[:, (CP - 1) * S : (CP - 1) * S + 1], 1.0)

                def combine(t0):
                    """tau slice [t0, t0+SLC): u, d, tree, sign -> dec."""
                    for gi in range(2):
                        n = CD if gi == 0 else CP
                        c0 = 0 if gi == 0 else CD
                        xs, rs, us, ds, t4s, t2s, es = (
                            (xD, rgD, uD, dcD, t4D, t2D, eD)
                            if gi == 0
                            else (xP, rgP, uP, dcP, t4P, t2P, eP)
                        )
                        r0 = t0 % RING
                        xv = xs[:, t0 * n * S : (t0 + SLC) * n * S]
                        rv = rs[:, r0 * n * S : (r0 + SLC) * n * S]
                        uv = us[:, : SLC * n * S]
                        (nc.vector if gi == 0 else nc.gpsimd).tensor_tensor(
                            uv, xv, rv, OP.mult
                        )
                        u4 = uv.rearrange(
                            "p (t c j d) -> p t c j d", t=SLC, c=n, d=2
                        )
                        dv = ds[:, : SLC * n * HS].rearrange(
                            "p (t c j) -> p t c j", t=SLC, j=HS
                        )
                        nc.gpsimd.tensor_tensor(
                            dv, u4[:, :, :, :, 0], u4[:, :, :, :, 1],
                            OP.subtract,
                        )
                        t4v = t4s[:, : SLC * n * 4].rearrange(
                            "p (t c j) -> p t c j", t=SLC, j=4
                        )
                        nc.vector.tensor_tensor(
                            t4v, dv[:, :, :, 0:4], dv[:, :, :, 4:8], OP.add
                        )
                        t2v = t2s[:, : SLC * n * 2].rearrange(
                            "p (t c j) -> p t c j", t=SLC, j=2
                        )
                        nc.vector.tensor_tensor(
                            t2v, t4v[:, :, :, 0:2], t4v[:, :, :, 2:4], OP.add
                        )
                        ev = es[:, : SLC * n].rearrange(
                            "p (t c) -> p t c", t=SLC
                        )
                        nc.vector.tensor_tensor(
                            ev, t2v[:, :, :, 0], t2v[:, :, :, 1], OP.add
                        )
                        dtc = dec[:, :].rearrange(
                            "p (c t) -> p t c", c=C, t=L
                        )[:, t0 : t0 + SLC, c0 : c0 + n]
                        nc.vector.tensor_scalar(
                            dtc, ev, 0.0, None, OP.is_lt
                        )

                for tau in range(L - 1, -1, -1):
                    for gi, (eng, c0, c1, sa, sb, pt) in enumerate(GRP):
                        if tau == L - 1:
                            src = v3((curD if gi == 0 else curP)[:, :], c1 - c0)
                        else:
                            src = ring_col(tau + 1, gi)
                        beta_add(eng, src, pt, c1 - c0)
                        beta_mult(
                            eng, pt, ring_col(tau, gi), g_view(tau, c0, c1),
                            c1 - c0,
                        )
                    if tau % SLC == 0:
                        combine(tau)

                nc.sync.dma_start(out_d[:, :], dec[:, :])
    return nc


def _legalize_multiwait(bir):
    """Split multi-wait engine instructions (walrus allows one sem wait)."""
    n = 0
    for fn in bir["functions"]:
        for blk in fn["blocks"]:
            newl = []
            for inst in blk["instructions"]:
                si = inst.get("sync_info") or {}
                waits = si.get("on_wait") or []
                eng = inst.get("engine")
                if len(waits) >= 2 and eng in (
                    "DVE", "Pool", "Activation", "PE", "SP",
                ):
                    for j, w in enumerate(waits):
                        carrier = {
                            "name": inst["name"] + f"-wc{j}",
                            "opcode": "EventSemaphore",
                            "engine": eng,
                            "ins": [],
                            "outs": [],
                            "sync_info": {"on_wait": [w], "on_update": []},
                        }
                        if "debug" in inst:
                            carrier["debug"] = inst["debug"]
                        newl.append(carrier)
                        n += 1
                    si["on_wait"] = []
                    inst["sync_info"] = si
                newl.append(inst)
            blk["instructions"] = newl
    return n


def _finalize(nc):
    import json as _json

    bir = _json.loads(nc.to_json_bytes())
    _legalize_multiwait(bir)
    bts = _json.dumps(bir).encode()
    nc.to_json_bytes = lambda: bts
    return nc


def _prep(y, h, snr):
    y = np.ascontiguousarray(np.asarray(y, dtype=np.float32))
    h = np.ascontiguousarray(np.asarray(h, dtype=np.float32))
    snr_f = float(np.asarray(snr))
    sigma = np.float32(10.0 ** (-snr_f / 10.0))
    bits = (np.arange(S)[:, None] >> np.arange(MEM - 1, -1, -1)) & 1
    syms = (1 - 2 * bits).astype(np.float32)
    sp = (syms @ h[:, ::-1].T).astype(np.float32)         # [S, V]
    spneg = -sp.T[np.arange(BPC) % V].astype(np.float32)  # [BPC, S]
    scale = np.float32(-1.0 / (2.0 * sigma * sigma))
    return y, spneg, scale


def kernel(y, h, snr):
    import concourse.bass as bass
    from concourse.bass_utils import run_bass_kernel_spmd

    y, spneg, scale = _prep(y, h, snr)

    nc = bass.Bass()
    _build(nc, T, scale)
    _finalize(nc)

    in_maps = [
        {
            "yin": np.ascontiguousarray(
                np.concatenate([y[c * BPC : (c + 1) * BPC], spneg], axis=1)
            ),
        }
        for c in range(NCORES)
    ]
    res = run_bass_kernel_spmd(nc, in_maps, core_ids=list(range(NCORES)))
    dec = np.concatenate([r["dec"] for r in res.results], axis=0)  # [B, T]

    out = np.zeros((B, T), np.float32)
    out[:, MEM - 1 :] = dec[:, : T - (MEM - 1)]
    return out


# revision 9
# speedup vs baseline: 1.1526x; 1.1526x over previous
"""Windowed BCJR detector kernel for Trainium2, 8-core batch-parallel.

Layout per core: 128 words on SBUF partitions.  Time axis split into
C = T/L chunks of L=32 steps, each warmed up with W=16 extra steps from
a uniform state (chunk 0 / last chunk get the exact delta init); all
chunks advance in lockstep so each recursion step is one wide vector op
pair instead of 2*T tiny ones.  Chunks are split DVE/Pool by a tunable
boundary so both engines run chains concurrently.

g = exp(scale*(y-sp)^2) in tau-major layout ((tau*C + c)*16 + s):
squares for tau [W, L) (needed first, by the alpha warmup) come from
DVE/Pool (diff into PSUM scratch, tensor_scalar pow back to SBUF);
squares for tau [0, W) from Act Square calls (per-state bias); exp runs
in tau-quarter blocks ordered q3 q4 q1 q2.  All chain/combine tiles
coexist with the g tiles in SBUF, so the alpha chain overlaps the exp
tail instead of serializing behind it.

Combine (u = x*beta, even-odd diff, tree-sum, sign) trails the beta
chain in 8-tau slices against a 16-slot beta ring.

States are bf16 without intra-window rescaling: decisions are invariant
to per-(word,t) positive scaling, and a window whose likelihood mass
underflows decodes 0 exactly like the reference's NaN cascade.
"""

import math
import sys

import numpy as np

sys.path.insert(0, "/opt/trn_rl_repo")

B, T, S, MEM, V = 1024, 2048, 16, 4, 4
NCORES = 8
BPC = B // NCORES   # 128 words per core
L = 32              # chunk length (t-steps)
W = 16              # warmup steps
SLC = 8             # combine slice (tau steps)
RING = 16           # beta ring slots
HS = S // 2
CDF = 30 / 64       # fraction of chunks on DVE


def _build(nc, Tn, g_scale, stop_after="all"):
    import concourse.bass as bass  # noqa: F401
    from concourse import mybir, tile
    from concourse.alu_op_type import AluOpType as OP
    from concourse.mybir import ActivationFunctionType as AF

    dt = mybir.dt
    C = Tn // L
    CD = max(1, int(round(C * CDF)))
    CP = C - CD
    assert CP >= 1

    yin_d = nc.dram_tensor("yin", [BPC, Tn], dt.float16, kind="ExternalInput")
    spn_d = nc.dram_tensor("spn", [BPC, S], dt.float32, kind="ExternalInput")
    out_d = nc.dram_tensor("dec", [BPC, Tn], dt.bfloat16, kind="ExternalOutput")

    tc_ctx = tile.TileContext(nc)
    with tc_ctx as tc:
        with tc.tile_pool(name="per", bufs=1) as per:
            spneg = per.tile([BPC, S], dt.float32, tag="spneg")
            zero1 = per.tile([BPC, 1], dt.float32, tag="zero1")
            gbuf = per.tile([BPC, Tn * S], dt.bfloat16, tag="g")
            dec = per.tile([BPC, Tn], dt.bfloat16, tag="dec")

            with tc.tile_pool(name="chA", bufs=1) as chA:
                xD = chA.tile([BPC, L * CD * S], dt.bfloat16, tag="xD")
                xP = chA.tile([BPC, L * CP * S], dt.bfloat16, tag="xP")
                stDA = chA.tile([BPC, CD * S], dt.bfloat16, tag="stDA")
                stDB = chA.tile([BPC, CD * S], dt.bfloat16, tag="stDB")
                stPA = chA.tile([BPC, CP * S], dt.bfloat16, tag="stPA")
                stPB = chA.tile([BPC, CP * S], dt.bfloat16, tag="stPB")
                pD = chA.tile([BPC, CD * HS], dt.bfloat16, tag="pD")
                pP = chA.tile([BPC, CP * HS], dt.bfloat16, tag="pP")

                # ---------- helpers ----------
                def v3(flat, n):
                    return flat[:, 0 : n * S].rearrange(
                        "p (c s) -> p c s", s=S
                    )

                def g_view(tau, c0, c1):
                    o = (tau * C + c0) * S
                    return v3(gbuf[:, o : o + (c1 - c0) * S], c1 - c0)

                def alpha_add(eng, src3, ptile, n):
                    pv = ptile[:, : n * HS].rearrange("p (c j) -> p c j", j=HS)
                    eng.tensor_tensor(
                        pv, src3[:, :, 0:HS], src3[:, :, HS:S], OP.add
                    )

                def alpha_mult(eng, ptile, dst3, gv, n):
                    pv = (
                        ptile[:, : n * HS]
                        .rearrange("p (c j) -> p c j", j=HS)
                        .unsqueeze(3)
                        .broadcast_to((BPC, n, HS, 2))
                    )
                    eng.tensor_tensor(
                        dst3.rearrange("p c (j d) -> p c j d", d=2),
                        pv,
                        gv.rearrange("p c (j d) -> p c j d", d=2),
                        OP.mult,
                    )

                def beta_add(eng, src3, ptile, n):
                    s4 = src3.rearrange("p c (j d) -> p c j d", d=2)
                    cv = ptile[:, : n * HS].rearrange("p (c j) -> p c j", j=HS)
                    eng.tensor_tensor(
                        cv, s4[:, :, :, 0], s4[:, :, :, 1], OP.add
                    )

                def beta_mult(eng, ptile, dst3, gv, n):
                    cv = (
                        ptile[:, : n * HS]
                        .rearrange("p (c j) -> p c j", j=HS)
                        .unsqueeze(2)
                        .broadcast_to((BPC, n, 2, HS))
                    )
                    eng.tensor_tensor(
                        dst3.rearrange("p c (d j) -> p c d j", d=2),
                        cv,
                        gv.rearrange("p c (d j) -> p c d j", d=2),
                        OP.mult,
                    )

                def x_col(tau, gi):
                    xs, n = (xD, CD) if gi == 0 else (xP, CP)
                    return v3(xs[:, tau * n * S : (tau + 1) * n * S], n)

                # ---------- g generation ----------
                with tc.tile_pool(name="gy", bufs=1) as gy:
                    ybuf = gy.tile([BPC, Tn], dt.float16, tag="y")
                    with tc.psum_pool(name="ps", bufs=1) as ps:
                        dtmpD = ps.tile([BPC, 2 * C * S], dt.float16, tag="dtD")
                        dtmpP = ps.tile([BPC, 2 * C * S], dt.float16, tag="dtP")
                        with tc.tile_pool(name="gsq", bufs=1) as gsq:
                            sq = gsq.tile([BPC, Tn * S], dt.float16, tag="sq")

                            nc.sync.dma_start(ybuf[:, :], yin_d[:, :])
                            nc.sync.dma_start(spneg[:, :], spn_d[:, :])
                            nc.vector.memset(zero1[:, :], 0.0)

                            def y_wc(w0, n):
                                return ybuf[:, 0 : C * L].rearrange(
                                    "p (c w) -> p w c", c=C, w=L
                                )[:, w0 : w0 + n, :]

                            def sq_dp(eng, dtmp, w0, w1):
                                for a in range(w0, w1, 2):
                                    b = min(a + 2, w1)
                                    n = b - a
                                    dv = dtmp[:, : n * C * S].rearrange(
                                        "p (w c s) -> p w c s", w=n, c=C, s=S
                                    )
                                    yv = (
                                        y_wc(a, n)
                                        .unsqueeze(3)
                                        .broadcast_to((BPC, n, C, S))
                                    )
                                    sv = (
                                        spneg[:, :]
                                        .unsqueeze(1)
                                        .unsqueeze(1)
                                        .broadcast_to((BPC, n, C, S))
                                    )
                                    eng.tensor_tensor(dv, yv, sv, OP.add)
                                    eng.tensor_scalar(
                                        sq[:, a * C * S : b * C * S],
                                        dtmp[:, : n * C * S],
                                        2.0, None, OP.pow,
                                    )

                            def sq_act(w0, w1):
                                n = w1 - w0
                                sq4 = sq[:, :].rearrange(
                                    "p (t c s) -> p t c s", c=C, s=S
                                )
                                for s in range(S):
                                    nc.scalar.activation(
                                        sq4[:, w0:w1, :, s],
                                        y_wc(w0, n),
                                        AF.Square,
                                        bias=spneg[:, s : s + 1],
                                        scale=1.0,
                                    )

                            # warmup-critical rows first, on DVE/Pool
                            HW2 = (L + W) // 2  # 24: DVE tau[16,24)
                            sq_dp(nc.vector, dtmpD, W, HW2)
                            sq_dp(nc.gpsimd, dtmpP, HW2, L)
                            sq_act(0, W)

                            QT = L // 4
                            for q in (2, 3, 0, 1):
                                o0 = q * QT * C * S
                                o1 = (q + 1) * QT * C * S
                                nc.scalar.activation(
                                    gbuf[:, o0:o1], sq[:, o0:o1], AF.Exp,
                                    bias=zero1[:, :], scale=float(g_scale),
                                )

                            if stop_after == "g":
                                nc.sync.dma_start(
                                    out_d[:, 0:S], stDA[:, 0:S]
                                )
                                nc.vector.memset(stDA[:, 0:S], 0.0)

                            # ---------- alpha ----------
                            GRP = [
                                (nc.vector, 0, CD, stDA, stDB, pD),
                                (nc.gpsimd, CD, C, stPA, stPB, pP),
                            ]
                            nc.vector.memset(stDA[:, :], 1.0)
                            nc.gpsimd.memset(stPA[:, :], 1.0)
                            flip = False
                            for w in range(W):
                                for eng, c0, c1, sa, sb, pt in GRP:
                                    cur, nxt = (sb, sa) if flip else (sa, sb)
                                    cc0 = max(c0, 1)
                                    lo = cc0 - c0
                                    alpha_add(
                                        eng, v3(cur[:, lo * S :], c1 - cc0),
                                        pt, c1 - cc0,
                                    )
                                    gv = g_view(L - W + w, cc0 - 1, c1 - 1)
                                    alpha_mult(
                                        eng, pt,
                                        v3(nxt[:, lo * S :], c1 - cc0),
                                        gv, c1 - cc0,
                                    )
                                flip = not flip
                            curD = stDB if flip else stDA
                            curP = stPB if flip else stPA
                            nc.vector.memset(curD[:, 0:S], 0.0)
                            nc.vector.memset(curD[:, 0:1], 1.0)
                            for tau in range(L):
                                for gi, (eng, c0, c1, sa, sb, pt) in enumerate(
                                    GRP
                                ):
                                    if tau == 0:
                                        src = v3(
                                            (curD if gi == 0 else curP)[:, :],
                                            c1 - c0,
                                        )
                                    else:
                                        src = x_col(tau - 1, gi)
                                    alpha_add(eng, src, pt, c1 - c0)
                                    alpha_mult(
                                        eng, pt, x_col(tau, gi),
                                        g_view(tau, c0, c1), c1 - c0,
                                    )

                if stop_after in ("g", "alpha"):
                    nc.sync.dma_start(out_d[:, 0:S], xD[:, 0:S])
                    return nc

                # ---------- beta + trailing combine ----------
                with tc.tile_pool(name="chB", bufs=1) as chB:
                    rgD = chB.tile([BPC, RING * CD * S], dt.bfloat16, tag="rD")
                    rgP = chB.tile([BPC, RING * CP * S], dt.bfloat16, tag="rP")
                    uD = chB.tile([BPC, SLC * CD * S], dt.bfloat16, tag="uD")
                    uP = chB.tile([BPC, SLC * CP * S], dt.bfloat16, tag="uP")
                    dcD = chB.tile([BPC, SLC * CD * HS], dt.bfloat16, tag="dcD")
                    dcP = chB.tile([BPC, SLC * CP * HS], dt.bfloat16, tag="dcP")
                    t4D = chB.tile([BPC, SLC * CD * 4], dt.bfloat16, tag="t4D")
                    t4P = chB.tile([BPC, SLC * CP * 4], dt.bfloat16, tag="t4P")
                    t2D = chB.tile([BPC, SLC * CD * 2], dt.bfloat16, tag="t2D")
                    t2P = chB.tile([BPC, SLC * CP * 2], dt.bfloat16, tag="t2P")
                    eD = chB.tile([BPC, SLC * CD], dt.bfloat16, tag="eD")
                    eP = chB.tile([BPC, SLC * CP], dt.bfloat16, tag="eP")

                    def ring_col(tau, gi):
                        r = tau % RING
                        rs, n = (rgD, CD) if gi == 0 else (rgP, CP)
                        return v3(rs[:, r * n * S : (r + 1) * n * S], n)

                    GRP = [
                        (nc.vector, 0, CD, stDA, stDB, pD),
                        (nc.gpsimd, CD, C, stPA, stPB, pP),
                    ]
                    nc.vector.memset(stDA[:, :], 1.0)
                    nc.gpsimd.memset(stPA[:, :], 1.0)
                    flip = False
                    for w in range(W):
                        for eng, c0, c1, sa, sb, pt in GRP:
                            cur, nxt = (sb, sa) if flip else (sa, sb)
                            cc1 = min(c1, C - 1)
                            beta_add(eng, v3(cur[:, :], cc1 - c0), pt, cc1 - c0)
                            gv = g_view(W - 1 - w, c0 + 1, cc1 + 1)
                            beta_mult(
                                eng, pt, v3(nxt[:, :], cc1 - c0), gv, cc1 - c0
                            )
                        flip = not flip
                    curD = stDB if flip else stDA
                    curP = stPB if flip else stPA
                    nc.gpsimd.memset(curP[:, (CP - 1) * S :], 0.0)
                    nc.gpsimd.memset(
                        curP[:, (CP - 1) * S : (CP - 1) * S + 1], 1.0
                    )

                    def combine(t0):
                        for gi in range(2):
                            n = CD if gi == 0 else CP
                            c0 = 0 if gi == 0 else CD
                            xs, rs, us, ds, t4s, t2s, es = (
                                (xD, rgD, uD, dcD, t4D, t2D, eD)
                                if gi == 0
                                else (xP, rgP, uP, dcP, t4P, t2P, eP)
                            )
                            r0 = t0 % RING
                            xv = xs[:, t0 * n * S : (t0 + SLC) * n * S]
                            rv = rs[:, r0 * n * S : (r0 + SLC) * n * S]
                            uv = us[:, : SLC * n * S]
                            (nc.vector if gi == 0 else nc.gpsimd).tensor_tensor(
                                uv, xv, rv, OP.mult
                            )
                            u4 = uv.rearrange(
                                "p (t c j d) -> p t c j d", t=SLC, c=n, d=2
                            )
                            dv = ds[:, : SLC * n * HS].rearrange(
                                "p (t c j) -> p t c j", t=SLC, j=HS
                            )
                            nc.gpsimd.tensor_tensor(
                                dv, u4[:, :, :, :, 0], u4[:, :, :, :, 1],
                                OP.subtract,
                            )
                            t4v = t4s[:, : SLC * n * 4].rearrange(
                                "p (t c j) -> p t c j", t=SLC, j=4
                            )
                            nc.vector.tensor_tensor(
                                t4v, dv[:, :, :, 0:4], dv[:, :, :, 4:8], OP.add
                            )
                            t2v = t2s[:, : SLC * n * 2].rearrange(
                                "p (t c j) -> p t c j", t=SLC, j=2
                            )
                            nc.vector.tensor_tensor(
                                t2v, t4v[:, :, :, 0:2], t4v[:, :, :, 2:4],
                                OP.add,
                            )
                            ev = es[:, : SLC * n].rearrange(
                                "p (t c) -> p t c", t=SLC
                            )
                            nc.vector.tensor_tensor(
                                ev, t2v[:, :, :, 0], t2v[:, :, :, 1], OP.add
                            )
                            dtc = dec[:, :].rearrange(
                                "p (c t) -> p t c", c=C, t=L
                            )[:, t0 : t0 + SLC, c0 : c0 + n]
                            nc.vector.tensor_scalar(
                                dtc, ev, 0.0, None, OP.is_lt
                            )

                    for tau in range(L - 1, -1, -1):
                        for gi, (eng, c0, c1, sa, sb, pt) in enumerate(GRP):
                            if tau == L - 1:
                                src = v3(
                                    (curD if gi == 0 else curP)[:, :], c1 - c0
                                )
                            else:
                                src = ring_col(tau + 1, gi)
                            beta_add(eng, src, pt, c1 - c0)
                            beta_mult(
                                eng, pt, ring_col(tau, gi),
                                g_view(tau, c0, c1), c1 - c0,
                            )
                        if tau % SLC == 0 and stop_after == "all":
                            combine(tau)

                    nc.sync.dma_start(out_d[:, :], dec[:, :])
    return nc


def _legalize_multiwait(bir):
    """Split multi-wait engine instructions (walrus allows one sem wait)."""
    n = 0
    for fn in bir["functions"]:
        for blk in fn["blocks"]:
            newl = []
            for inst in blk["instructions"]:
                si = inst.get("sync_info") or {}
                waits = si.get("on_wait") or []
                eng = inst.get("engine")
                if len(waits) >= 2 and eng in (
                    "DVE", "Pool", "Activation", "PE", "SP",
                ):
                    for j, w in enumerate(waits):
                        carrier = {
                            "name": inst["name"] + f"-wc{j}",
                            "opcode": "EventSemaphore",
                            "engine": eng,
                            "ins": [],
                            "outs": [],
                            "sync_info": {"on_wait": [w], "on_update": []},
                        }
                        if "debug" in inst:
                            carrier["debug"] = inst["debug"]
                        newl.append(carrier)
                        n += 1
                    si["on_wait"] = []
                    inst["sync_info"] = si
                newl.append(inst)
            blk["instructions"] = newl
    return n


def _finalize(nc):
    import json as _json

    bir = _json.loads(nc.to_json_bytes())
    _legalize_multiwait(bir)
    bts = _json.dumps(bir).encode()
    nc.to_json_bytes = lambda: bts
    return nc


def _prep(y, h, snr):
    y = np.ascontiguousarray(np.asarray(y, dtype=np.float32))
    h = np.ascontiguousarray(np.asarray(h, dtype=np.float32))
    snr_f = float(np.asarray(snr))
    sigma = np.float32(10.0 ** (-snr_f / 10.0))
    bits = (np.arange(S)[:, None] >> np.arange(MEM - 1, -1, -1)) & 1
    syms = (1 - 2 * bits).astype(np.float32)
    sp = (syms @ h[:, ::-1].T).astype(np.float32)         # [S, V]
    spneg = -sp.T[np.arange(BPC) % V].astype(np.float32)  # [BPC, S]
    scale = np.float32(-1.0 / (2.0 * sigma * sigma))
    return y, spneg, scale


def kernel(y, h, snr):
    import concourse.bass as bass
    from concourse.bass_utils import run_bass_kernel_spmd

    y, spneg, scale = _prep(y, h, snr)
    y16 = y.astype(np.float16)

    nc = bass.Bass()
    _build(nc, T, scale)
    _finalize(nc)

    in_maps = [
        {
            "yin": np.ascontiguousarray(y16[c * BPC : (c + 1) * BPC]),
            "spn": spneg,
        }
        for c in range(NCORES)
    ]
    res = run_bass_kernel_spmd(nc, in_maps, core_ids=list(range(NCORES)))
    dec = np.concatenate(
        [np.asarray(r["dec"]).astype(np.float32) for r in res.results], axis=0
    )

    out = np.zeros((B, T), np.float32)
    out[:, MEM - 1 :] = dec[:, : T - (MEM - 1)]
    return out


# revision 13
# speedup vs baseline: 4.2854x; 3.7179x over previous
"""Windowed BCJR detector kernel for Trainium2, 8-core batch-parallel.

Layout per core: 128 words on SBUF partitions.  Time axis split into
C = T/L chunks of L=32 steps, each warmed up with W=16 extra steps from
a uniform state (chunk 0 / last chunk get the exact delta init); all
chunks advance in lockstep so each recursion step is one wide vector op
pair instead of 2*T tiny ones.  Chunks are split DVE/Pool by a tunable
boundary so both engines run chains concurrently.

g = exp(scale*(y-sp)^2) in tau-major layout ((tau*C + c)*16 + s):
squares for tau [W, L) (needed first, by the alpha warmup) come from
DVE/Pool (diff into PSUM scratch, tensor_scalar pow back to SBUF);
squares for tau [0, W) from Act Square calls (per-state bias); exp runs
in tau-quarter blocks ordered q3 q4 q1 q2.  All chain/combine tiles
coexist with the g tiles in SBUF, so the alpha chain overlaps the exp
tail instead of serializing behind it.

Combine (u = x*beta, even-odd diff, tree-sum, sign) trails the beta
chain in 8-tau slices against a 16-slot beta ring.

States are bf16 without intra-window rescaling: decisions are invariant
to per-(word,t) positive scaling, and a window whose likelihood mass
underflows decodes 0 exactly like the reference's NaN cascade.
"""

import math
import sys

import numpy as np

sys.path.insert(0, "/opt/trn_rl_repo")

B, T, S, MEM, V = 1024, 2048, 16, 4, 4
NCORES = 8
BPC = B // NCORES   # 128 words per core
L = 32              # chunk length (t-steps)
W = 16              # warmup steps
SLC = 16            # combine slice (tau steps)
RING = 16           # beta ring slots
HS = S // 2
CDF = 31 / 64       # fraction of chunks on DVE


def _build(nc, Tn, g_scale, stop_after="all"):
    import concourse.bass as bass  # noqa: F401
    from concourse import mybir, tile
    from concourse.alu_op_type import AluOpType as OP
    from concourse.mybir import ActivationFunctionType as AF

    dt = mybir.dt
    C = Tn // L
    CD = max(1, int(round(C * CDF)))
    CP = C - CD
    assert CP >= 1

    yin_d = nc.dram_tensor("yin", [BPC, Tn], dt.float16, kind="ExternalInput")
    spn_d = nc.dram_tensor("spn", [BPC, S], dt.float32, kind="ExternalInput")
    out_d = nc.dram_tensor("dec", [BPC, Tn], dt.bfloat16, kind="ExternalOutput")

    tc_ctx = tile.TileContext(nc)
    with tc_ctx as tc:
        with tc.tile_pool(name="per", bufs=1) as per:
            spneg = per.tile([BPC, S], dt.float32, tag="spneg")
            zero1 = per.tile([BPC, 1], dt.float32, tag="zero1")
            gbuf = per.tile([BPC, Tn * S], dt.bfloat16, tag="g")
            dec = per.tile([BPC, Tn], dt.bfloat16, tag="dec")

            with tc.tile_pool(name="chA", bufs=1) as chA:
                xD = chA.tile([BPC, L * CD * S], dt.bfloat16, tag="xD")
                xP = chA.tile([BPC, L * CP * S], dt.bfloat16, tag="xP")
                stDA = chA.tile([BPC, CD * S], dt.bfloat16, tag="stDA")
                stDB = chA.tile([BPC, CD * S], dt.bfloat16, tag="stDB")
                stPA = chA.tile([BPC, CP * S], dt.bfloat16, tag="stPA")
                stPB = chA.tile([BPC, CP * S], dt.bfloat16, tag="stPB")
                pD = chA.tile([BPC, CD * HS], dt.bfloat16, tag="pD")
                pP = chA.tile([BPC, CP * HS], dt.bfloat16, tag="pP")

                # ---------- helpers ----------
                def v3(flat, n):
                    return flat[:, 0 : n * S].rearrange(
                        "p (c s) -> p c s", s=S
                    )

                def g_view(tau, c0, c1):
                    o = (tau * C + c0) * S
                    return v3(gbuf[:, o : o + (c1 - c0) * S], c1 - c0)

                def alpha_add(eng, src3, ptile, n):
                    pv = ptile[:, : n * HS].rearrange("p (c j) -> p c j", j=HS)
                    eng.tensor_tensor(
                        pv, src3[:, :, 0:HS], src3[:, :, HS:S], OP.add
                    )

                def alpha_mult(eng, ptile, dst3, gv, n):
                    pv = (
                        ptile[:, : n * HS]
                        .rearrange("p (c j) -> p c j", j=HS)
                        .unsqueeze(3)
                        .broadcast_to((BPC, n, HS, 2))
                    )
                    eng.tensor_tensor(
                        dst3.rearrange("p c (j d) -> p c j d", d=2),
                        pv,
                        gv.rearrange("p c (j d) -> p c j d", d=2),
                        OP.mult,
                    )

                def beta_add(eng, src3, ptile, n):
                    s4 = src3.rearrange("p c (j d) -> p c j d", d=2)
                    cv = ptile[:, : n * HS].rearrange("p (c j) -> p c j", j=HS)
                    eng.tensor_tensor(
                        cv, s4[:, :, :, 0], s4[:, :, :, 1], OP.add
                    )

                def beta_mult(eng, ptile, dst3, gv, n):
                    cv = (
                        ptile[:, : n * HS]
                        .rearrange("p (c j) -> p c j", j=HS)
                        .unsqueeze(2)
                        .broadcast_to((BPC, n, 2, HS))
                    )
                    eng.tensor_tensor(
                        dst3.rearrange("p c (d j) -> p c d j", d=2),
                        cv,
                        gv.rearrange("p c (d j) -> p c d j", d=2),
                        OP.mult,
                    )

                def x_col(tau, gi):
                    xs, n = (xD, CD) if gi == 0 else (xP, CP)
                    return v3(xs[:, tau * n * S : (tau + 1) * n * S], n)

                # ---------- g generation ----------
                with tc.tile_pool(name="gy", bufs=1) as gy:
                    ybuf = gy.tile([BPC, Tn], dt.float16, tag="y")
                    with tc.psum_pool(name="ps", bufs=1) as ps:
                        dtmpD = ps.tile([BPC, 2 * C * S], dt.float16, tag="dtD")
                        dtmpP = ps.tile([BPC, 2 * C * S], dt.float16, tag="dtP")
                        with tc.tile_pool(name="gsq", bufs=1) as gsq:
                            sq = gsq.tile([BPC, Tn * S], dt.float16, tag="sq")

                            nc.sync.dma_start(ybuf[:, :], yin_d[:, :])
                            nc.sync.dma_start(spneg[:, :], spn_d[:, :])
                            nc.vector.memset(zero1[:, :], 0.0)

                            def y_wc(w0, n):
                                return ybuf[:, 0 : C * L].rearrange(
                                    "p (c w) -> p w c", c=C, w=L
                                )[:, w0 : w0 + n, :]

                            def sq_dp(eng, dtmp, w0, w1):
                                for a in range(w0, w1, 2):
                                    b = min(a + 2, w1)
                                    n = b - a
                                    dv = dtmp[:, : n * C * S].rearrange(
                                        "p (w c s) -> p w c s", w=n, c=C, s=S
                                    )
                                    yv = (
                                        y_wc(a, n)
                                        .unsqueeze(3)
                                        .broadcast_to((BPC, n, C, S))
                                    )
                                    sv = (
                                        spneg[:, :]
                                        .unsqueeze(1)
                                        .unsqueeze(1)
                                        .broadcast_to((BPC, n, C, S))
                                    )
                                    eng.tensor_tensor(dv, yv, sv, OP.add)
                                    eng.tensor_scalar(
                                        sq[:, a * C * S : b * C * S],
                                        dtmp[:, : n * C * S],
                                        2.0, None, OP.pow,
                                    )

                            def sq_act(w0, w1):
                                n = w1 - w0
                                sq4 = sq[:, :].rearrange(
                                    "p (t c s) -> p t c s", c=C, s=S
                                )
                                for s in range(S):
                                    nc.scalar.activation(
                                        sq4[:, w0:w1, :, s],
                                        y_wc(w0, n),
                                        AF.Square,
                                        bias=spneg[:, s : s + 1],
                                        scale=1.0,
                                    )

                            # warmup-critical rows first, on DVE/Pool
                            HW2 = (L + W) // 2  # 24: DVE tau[16,24)
                            sq_dp(nc.vector, dtmpD, W, HW2)
                            sq_dp(nc.gpsimd, dtmpP, HW2, L)
                            sq_act(0, W)

                            QT = L // 4
                            for q in (2, 3, 0, 1):
                                o0 = q * QT * C * S
                                o1 = (q + 1) * QT * C * S
                                nc.scalar.activation(
                                    gbuf[:, o0:o1], sq[:, o0:o1], AF.Exp,
                                    bias=zero1[:, :], scale=float(g_scale),
                                )

                            if stop_after == "g":
                                nc.sync.dma_start(
                                    out_d[:, 0:S], stDA[:, 0:S]
                                )
                                nc.vector.memset(stDA[:, 0:S], 0.0)

                            # ---------- alpha ----------
                            GRP = [
                                (nc.vector, 0, CD, stDA, stDB, pD),
                                (nc.gpsimd, CD, C, stPA, stPB, pP),
                            ]
                            nc.vector.memset(stDA[:, :], 1.0)
                            nc.gpsimd.memset(stPA[:, :], 1.0)
                            flip = False
                            for w in range(W):
                                for eng, c0, c1, sa, sb, pt in GRP:
                                    cur, nxt = (sb, sa) if flip else (sa, sb)
                                    cc0 = max(c0, 1)
                                    lo = cc0 - c0
                                    alpha_add(
                                        eng, v3(cur[:, lo * S :], c1 - cc0),
                                        pt, c1 - cc0,
                                    )
                                    gv = g_view(L - W + w, cc0 - 1, c1 - 1)
                                    alpha_mult(
                                        eng, pt,
                                        v3(nxt[:, lo * S :], c1 - cc0),
                                        gv, c1 - cc0,
                                    )
                                flip = not flip
                            curD = stDB if flip else stDA
                            curP = stPB if flip else stPA
                            nc.vector.memset(curD[:, 0:S], 0.0)
                            nc.vector.memset(curD[:, 0:1], 1.0)
                            for tau in range(L):
                                for gi, (eng, c0, c1, sa, sb, pt) in enumerate(
                                    GRP
                                ):
                                    if tau == 0:
                                        src = v3(
                                            (curD if gi == 0 else curP)[:, :],
                                            c1 - c0,
                                        )
                                    else:
                                        src = x_col(tau - 1, gi)
                                    alpha_add(eng, src, pt, c1 - c0)
                                    alpha_mult(
                                        eng, pt, x_col(tau, gi),
                                        g_view(tau, c0, c1), c1 - c0,
                                    )

                if stop_after in ("g", "alpha"):
                    nc.sync.dma_start(out_d[:, 0:S], xD[:, 0:S])
                    return nc

                # ---------- beta + trailing combine ----------
                with tc.tile_pool(name="chB", bufs=1) as chB:
                    rgD = chB.tile([BPC, RING * CD * S], dt.bfloat16, tag="rD")
                    rgP = chB.tile([BPC, RING * CP * S], dt.bfloat16, tag="rP")
                    uD = chB.tile([BPC, SLC * CD * S], dt.bfloat16, tag="uD")
                    uP = chB.tile([BPC, SLC * CP * S], dt.bfloat16, tag="uP")
                    dcD = chB.tile([BPC, SLC * CD * HS], dt.bfloat16, tag="dcD")
                    dcP = chB.tile([BPC, SLC * CP * HS], dt.bfloat16, tag="dcP")
                    t4D = chB.tile([BPC, SLC * CD * 4], dt.bfloat16, tag="t4D")
                    t4P = chB.tile([BPC, SLC * CP * 4], dt.bfloat16, tag="t4P")
                    t2D = chB.tile([BPC, SLC * CD * 2], dt.bfloat16, tag="t2D")
                    t2P = chB.tile([BPC, SLC * CP * 2], dt.bfloat16, tag="t2P")
                    eD = chB.tile([BPC, SLC * CD], dt.bfloat16, tag="eD")
                    eP = chB.tile([BPC, SLC * CP], dt.bfloat16, tag="eP")

                    def ring_col(tau, gi):
                        r = tau % RING
                        rs, n = (rgD, CD) if gi == 0 else (rgP, CP)
                        return v3(rs[:, r * n * S : (r + 1) * n * S], n)

                    GRP = [
                        (nc.vector, 0, CD, stDA, stDB, pD),
                        (nc.gpsimd, CD, C, stPA, stPB, pP),
                    ]
                    nc.vector.memset(stDA[:, :], 1.0)
                    nc.gpsimd.memset(stPA[:, :], 1.0)
                    flip = False
                    for w in range(W):
                        for eng, c0, c1, sa, sb, pt in GRP:
                            cur, nxt = (sb, sa) if flip else (sa, sb)
                            cc1 = min(c1, C - 1)
                            beta_add(eng, v3(cur[:, :], cc1 - c0), pt, cc1 - c0)
                            gv = g_view(W - 1 - w, c0 + 1, cc1 + 1)
                            beta_mult(
                                eng, pt, v3(nxt[:, :], cc1 - c0), gv, cc1 - c0
                            )
                        flip = not flip
                    curD = stDB if flip else stDA
                    curP = stPB if flip else stPA
                    nc.gpsimd.memset(curP[:, (CP - 1) * S :], 0.0)
                    nc.gpsimd.memset(
                        curP[:, (CP - 1) * S : (CP - 1) * S + 1], 1.0
                    )

                    def combine(t0):
                        for gi in range(2):
                            n = CD if gi == 0 else CP
                            c0 = 0 if gi == 0 else CD
                            xs, rs, us, ds, t4s, t2s, es = (
                                (xD, rgD, uD, dcD, t4D, t2D, eD)
                                if gi == 0
                                else (xP, rgP, uP, dcP, t4P, t2P, eP)
                            )
                            r0 = t0 % RING
                            xv = xs[:, t0 * n * S : (t0 + SLC) * n * S]
                            rv = rs[:, r0 * n * S : (r0 + SLC) * n * S]
                            uv = us[:, : SLC * n * S]
                            # balance: u-mults packed -> DVE; d-subs
                            # (stride-2, unpacked) -> Pool; trees split
                            nc.vector.tensor_tensor(uv, xv, rv, OP.mult)
                            deng = nc.gpsimd
                            teng = nc.vector if gi == 0 else nc.gpsimd
                            u4 = uv.rearrange(
                                "p (t c j d) -> p t c j d", t=SLC, c=n, d=2
                            )
                            dv = ds[:, : SLC * n * HS].rearrange(
                                "p (t c j) -> p t c j", t=SLC, j=HS
                            )
                            deng.tensor_tensor(
                                dv, u4[:, :, :, :, 0], u4[:, :, :, :, 1],
                                OP.subtract,
                            )
                            t4v = t4s[:, : SLC * n * 4].rearrange(
                                "p (t c j) -> p t c j", t=SLC, j=4
                            )
                            teng.tensor_tensor(
                                t4v, dv[:, :, :, 0:4], dv[:, :, :, 4:8], OP.add
                            )
                            t2v = t2s[:, : SLC * n * 2].rearrange(
                                "p (t c j) -> p t c j", t=SLC, j=2
                            )
                            teng.tensor_tensor(
                                t2v, t4v[:, :, :, 0:2], t4v[:, :, :, 2:4],
                                OP.add,
                            )
                            ev = es[:, : SLC * n].rearrange(
                                "p (t c) -> p t c", t=SLC
                            )
                            teng.tensor_tensor(
                                ev, t2v[:, :, :, 0], t2v[:, :, :, 1], OP.add
                            )
                            dtc = dec[:, :].rearrange(
                                "p (c t) -> p t c", c=C, t=L
                            )[:, t0 : t0 + SLC, c0 : c0 + n]
                            teng.tensor_scalar(
                                dtc, ev, 0.0, None, OP.is_lt
                            )

                    for tau in range(L - 1, -1, -1):
                        for gi, (eng, c0, c1, sa, sb, pt) in enumerate(GRP):
                            if tau == L - 1:
                                src = v3(
                                    (curD if gi == 0 else curP)[:, :], c1 - c0
                                )
                            else:
                                src = ring_col(tau + 1, gi)
                            beta_add(eng, src, pt, c1 - c0)
                            beta_mult(
                                eng, pt, ring_col(tau, gi),
                                g_view(tau, c0, c1), c1 - c0,
                            )
                        if tau % SLC == 0 and stop_after == "all":
                            combine(tau)

                    nc.sync.dma_start(out_d[:, :], dec[:, :])
    return nc


def _legalize_multiwait(bir):
    """Split multi-wait engine instructions (walrus allows one sem wait)."""
    n = 0
    for fn in bir["functions"]:
        for blk in fn["blocks"]:
            newl = []
            for inst in blk["instructions"]:
                si = inst.get("sync_info") or {}
                waits = si.get("on_wait") or []
                eng = inst.get("engine")
                if len(waits) >= 2 and eng in (
                    "DVE", "Pool", "Activation", "PE", "SP",
                ):
                    for j, w in enumerate(waits):
                        carrier = {
                            "name": inst["name"] + f"-wc{j}",
                            "opcode": "EventSemaphore",
                            "engine": eng,
                            "ins": [],
                            "outs": [],
                            "sync_info": {"on_wait": [w], "on_update": []},
                        }
                        if "debug" in inst:
                            carrier["debug"] = inst["debug"]
                        newl.append(carrier)
                        n += 1
                    si["on_wait"] = []
                    inst["sync_info"] = si
                newl.append(inst)
            blk["instructions"] = newl
    return n


def _finalize(nc):
    import json as _json

    bir = _json.loads(nc.to_json_bytes())
    _legalize_multiwait(bir)
    bts = _json.dumps(bir).encode()
    nc.to_json_bytes = lambda: bts
    return nc


def _prep(y, h, snr):
    y = np.ascontiguousarray(np.asarray(y, dtype=np.float32))
    h = np.ascontiguousarray(np.asarray(h, dtype=np.float32))
    snr_f = float(np.asarray(snr))
    sigma = np.float32(10.0 ** (-snr_f / 10.0))
    bits = (np.arange(S)[:, None] >> np.arange(MEM - 1, -1, -1)) & 1
    syms = (1 - 2 * bits).astype(np.float32)
    sp = (syms @ h[:, ::-1].T).astype(np.float32)         # [S, V]
    spneg = -sp.T[np.arange(BPC) % V].astype(np.float32)  # [BPC, S]
    scale = np.float32(-1.0 / (2.0 * sigma * sigma))
    return y, spneg, scale


def kernel(y, h, snr):
    import concourse.bass as bass
    from concourse.bass_utils import run_bass_kernel_spmd

    y, spneg, scale = _prep(y, h, snr)
    y16 = y.astype(np.float16)

    nc = bass.Bass()
    _build(nc, T, scale)
    _finalize(nc)

    in_maps = [
        {
            "yin": np.ascontiguousarray(y16[c * BPC : (c + 1) * BPC]),
            "spn": spneg,
        }
        for c in range(NCORES)
    ]
    res = run_bass_kernel_spmd(nc, in_maps, core_ids=list(range(NCORES)))
    dec = np.concatenate(
        [np.asarray(r["dec"]).astype(np.float32) for r in res.results], axis=0
    )

    out = np.zeros((B, T), np.float32)
    out[:, MEM - 1 :] = dec[:, : T - (MEM - 1)]
    return out
